# revision 7
# baseline (speedup 1.0000x reference)
"""Bilateral filter (5x5, sigma_space=1.5, sigma_color=0.1) on Trainium2.

Full input img [2,3,1024,1024] f32 -> full output, same shape.
Sharding: each of 8 cores takes a 128-row horizontal band of all 6 (B*C)
images, with a 2-row halo on each side (reflect-padded host-side). W is
also reflect-padded host-side to 1028 so the kernel has zero boundary
logic.

Math per pixel x (25 taps t of the 5x5 window):
    w_t = k2_t * exp(-50*(p(x+t)-p(x))^2)     [k2 = outer 5-tap gaussian]
    out = sum_t w_t * p(x+t) / (sum_t w_t + 1e-8)

In-kernel layout: H rows on partitions, W on the free dim. The 5
partition-shift variants (dh in -2..2) are realized as 5 overlapping DMA
loads from the halo'd shard; dw shifts are free-dim AP offsets.
"""

import math
import sys

try:
    import concourse.bass  # noqa: F401
except ImportError:  # fresh env without PYTHONPATH — use the repo copy
    sys.path.insert(0, "/opt/trn_rl_repo")

import numpy as np

import concourse.bass as bass
import concourse.mybir as mybir
from concourse import bass_utils
from concourse.tile import TileContext

# -------------------------------------------------------------- constants
KS = 5
PAD = KS // 2
SIGMA = 1.5
SIGMA_COLOR = 0.1
EPS = 1e-8
N_CORES = 8
B, C, H, W = 2, 3, 1024, 1024
NIMG = B * C
BAND = H // N_CORES          # 128 output rows per core
SH_H = BAND + 2 * PAD        # 132 input rows per core shard
SH_W = W + 2 * PAD           # 1028 input cols

F32 = mybir.dt.float32


def _gauss_k2() -> np.ndarray:
    x = np.arange(KS, dtype=np.float32) - KS // 2
    k1 = np.exp(-0.5 * (x / np.float32(SIGMA)) ** 2).astype(np.float32)
    k1 = k1 / k1.sum(dtype=np.float32)
    return np.outer(k1, k1).astype(np.float32)  # [5,5]


def build_nc(split_waits: bool = True) -> bass.Bass:
    k2 = _gauss_k2()
    k2c = float(k2[PAD, PAD])
    neg_half_inv_s2 = -0.5 / (SIGMA_COLOR * SIGMA_COLOR)  # -50.0

    nc = bass.Bass()
    x = nc.dram_tensor("x", [NIMG, SH_H, SH_W], F32, kind="ExternalInput")
    y = nc.dram_tensor("y", [NIMG, BAND, W], F32, kind="ExternalOutput")

    mult = mybir.AluOpType.mult
    Exp = mybir.ActivationFunctionType.Exp

    with TileContext(nc) as tc:
        with (
            tc.tile_pool(name="pconst", bufs=1) as pconst,
            tc.tile_pool(name="pin", bufs=2) as pin,
            tc.tile_pool(name="pacc", bufs=2) as pacc,
            tc.tile_pool(name="ptmp", bufs=3) as ptmp,
        ):
            # per-tap activation biases ln(k2): [128, 25] column table
            btab = pconst.tile([128, KS * KS], F32, tag="btab")
            for j in range(KS):
                for i in range(KS):
                    nc.vector.memset(
                        btab[:, j * KS + i : j * KS + i + 1],
                        float(math.log(k2[j, i])),
                    )
            for im in range(NIMG):
                # 5 partition-shifted copies: ps[j][p,:] = x[im, p+j, :]
                # output row r lives at partition r; shard row r+2 == center.
                ps = []
                for j in range(KS):  # dh = j - 2
                    t = pin.tile([BAND, SH_W], F32, tag=f"ps{j}")
                    nc.sync.dma_start(out=t[:], in_=x[im, j : j + BAND, :])
                    ps.append(t)
                ctr = ps[PAD][:, PAD : PAD + W]

                den = pacc.tile([BAND, W], F32, tag="den")
                num = pacc.tile([BAND, W], F32, tag="num")
                nc.vector.memset(den[:], k2c + EPS)
                nc.vector.tensor_scalar_mul(num[:], ctr, k2c)

                for j in range(KS):
                    for i in range(KS):
                        dh, dw = j - PAD, i - PAD
                        if dh == 0 and dw == 0:
                            continue
                        src = ps[j][:, i : i + W]
                        d = ptmp.tile([BAND, W], F32, tag="d")
                        s = ptmp.tile([BAND, W], F32, tag="s")
                        w = ptmp.tile([BAND, W], F32, tag="w")
                        m = ptmp.tile([BAND, W], F32, tag="m")
                        nc.vector.tensor_sub(d[:], src, ctr)
                        nc.vector.scalar_tensor_tensor(
                            s[:], d[:], neg_half_inv_s2, d[:], mult, mult
                        )
                        nc.scalar.activation(
                            w[:], s[:], Exp,
                            bias=btab[:, j * KS + i : j * KS + i + 1],
                        )
                        nc.vector.tensor_mul(m[:], w[:], src)
                        nc.vector.tensor_add(num[:], num[:], m[:])
                        nc.vector.tensor_add(den[:], den[:], w[:])

                r = pacc.tile([BAND, W], F32, tag="r")
                o = pacc.tile([BAND, W], F32, tag="o")
                nc.vector.reciprocal(r[:], den[:])
                nc.vector.tensor_mul(o[:], num[:], r[:])
                nc.sync.dma_start(out=y[im], in_=o[:])

    if split_waits:
        _split_drain_waits(nc)
    return nc


def _split_drain_waits(nc: bass.Bass, maxw: int = 1) -> int:
    """This walrus build caps EVERY instruction at 1 sync wait. Tile attaches
    several. Split excess waits onto inserted same-engine Drain carriers
    (sequential on one engine, so semantics are identical)."""
    n_new = 0
    for f in nc.m.functions:
        for b in f.blocks:
            newlist = []
            for ins in b.instructions:
                si = ins.sync_info
                waits = list(si.on_wait) if si and si.on_wait else []
                if len(waits) > maxw:
                    head = waits[:-maxw]
                    while head:
                        chunk, head = head[:maxw], head[maxw:]
                        n_new += 1
                        pre = mybir.InstDrain(
                            name=f"{ins.name}_ws{n_new}",
                            ins=[],
                            outs=[],
                            sync_info=mybir.SyncInfo(on_wait=chunk, on_update=[]),
                        )
                        pre.engine = ins.engine
                        newlist.append(pre)
                    si.on_wait = waits[-maxw:]
                newlist.append(ins)
            b.instructions = newlist
    return n_new


def _shard_inputs(img: np.ndarray) -> list[dict[str, np.ndarray]]:
    pd = np.pad(
        img.astype(np.float32, copy=False),
        ((0, 0), (0, 0), (PAD, PAD), (PAD, PAD)),
        mode="reflect",
    ).reshape(NIMG, H + 2 * PAD, SH_W)
    return [
        {"x": np.ascontiguousarray(pd[:, BAND * c : BAND * c + SH_H, :])}
        for c in range(N_CORES)
    ]


_NC_CACHE: list = []


def kernel(img: np.ndarray) -> np.ndarray:
    img = np.asarray(img)
    assert img.shape == (B, C, H, W), img.shape
    if not _NC_CACHE:
        _NC_CACHE.append(build_nc())
    nc = _NC_CACHE[0]
    in_maps = _shard_inputs(img)
    res = bass_utils.run_bass_kernel_spmd(nc, in_maps, core_ids=list(range(N_CORES)))
    full = np.concatenate([r["y"] for r in res.results], axis=1)  # [6,1024,1024]
    return full.reshape(B, C, H, W).astype(img.dtype, copy=False)


# revision 9
# speedup vs baseline: 1.2745x; 1.2745x over previous
"""Bilateral filter (5x5, sigma_space=1.5, sigma_color=0.1) on Trainium2.

Full input img [2,3,1024,1024] f32 -> full output, same shape.
Sharding: each of 8 cores takes a 128-row horizontal band of all 6 (B*C)
images, with a 2-row halo on each side (reflect-padded host-side). W is
also reflect-padded host-side to 1028 so the kernel has zero boundary
logic.

Math per pixel x (25 taps t of the 5x5 window):
    w_t = k2_t * exp(-50*(p(x+t)-p(x))^2)     [k2 = outer 5-tap gaussian]
    out = sum_t w_t * p(x+t) / (sum_t w_t + 1e-8)

In-kernel layout: H rows on partitions, W on the free dim. The 5
partition-shift variants (dh in -2..2) are realized as 5 overlapping DMA
loads from the halo'd shard; dw shifts are free-dim AP offsets.
"""

import math
import sys

try:
    import concourse.bass  # noqa: F401
except ImportError:  # fresh env without PYTHONPATH — use the repo copy
    sys.path.insert(0, "/opt/trn_rl_repo")

import numpy as np

import concourse.bass as bass
import concourse.mybir as mybir
from concourse import bass_utils
from concourse.tile import TileContext

# -------------------------------------------------------------- constants
KS = 5
PAD = KS // 2
SIGMA = 1.5
SIGMA_COLOR = 0.1
EPS = 1e-8
N_CORES = 8
B, C, H, W = 2, 3, 1024, 1024
NIMG = B * C
BAND = H // N_CORES          # 128 output rows per core
SH_H = BAND + 2 * PAD        # 132 input rows per core shard
SH_W = W + 2 * PAD           # 1028 input cols

F32 = mybir.dt.float32


def _gauss_k2() -> np.ndarray:
    x = np.arange(KS, dtype=np.float32) - KS // 2
    k1 = np.exp(-0.5 * (x / np.float32(SIGMA)) ** 2).astype(np.float32)
    k1 = k1 / k1.sum(dtype=np.float32)
    return np.outer(k1, k1).astype(np.float32)  # [5,5]


def build_nc(split_waits: bool = True) -> bass.Bass:
    k2 = _gauss_k2()
    k2c = float(k2[PAD, PAD])
    neg_half_inv_s2 = -0.5 / (SIGMA_COLOR * SIGMA_COLOR)  # -50.0

    nc = bass.Bass()
    x = nc.dram_tensor("x", [NIMG, SH_H, SH_W], F32, kind="ExternalInput")
    y = nc.dram_tensor("y", [NIMG, BAND, W], F32, kind="ExternalOutput")

    Exp = mybir.ActivationFunctionType.Exp
    Square = mybir.ActivationFunctionType.Square
    sqrt_half_inv_s2 = math.sqrt(-neg_half_inv_s2)  # sqrt(50)

    with TileContext(nc) as tc:
        with (
            tc.tile_pool(name="pconst", bufs=1) as pconst,
            tc.tile_pool(name="pin", bufs=2) as pin,
            tc.tile_pool(name="pacc", bufs=2) as pacc,
            tc.tile_pool(name="ptmp", bufs=3) as ptmp,
        ):
            # per-tap activation biases ln(k2): [128, 25] column table
            btab = pconst.tile([128, KS * KS], F32, tag="btab")
            for j in range(KS):
                for i in range(KS):
                    nc.vector.memset(
                        btab[:, j * KS + i : j * KS + i + 1],
                        float(math.log(k2[j, i])),
                    )
            for im in range(NIMG):
                # 5 partition-shifted copies: ps[j][p,:] = x[im, p+j, :]
                # output row r lives at partition r; shard row r+2 == center.
                ps = []
                for j in range(KS):  # dh = j - 2
                    t = pin.tile([BAND, SH_W], F32, tag=f"ps{j}")
                    nc.sync.dma_start(out=t[:], in_=x[im, j : j + BAND, :])
                    ps.append(t)
                ctr = ps[PAD][:, PAD : PAD + W]

                den = pacc.tile([BAND, W], F32, tag="den")
                num = pacc.tile([BAND, W], F32, tag="num")
                nc.vector.memset(den[:], k2c + EPS)
                nc.vector.tensor_scalar_mul(num[:], ctr, k2c)

                for j in range(KS):
                    for i in range(KS):
                        dh, dw = j - PAD, i - PAD
                        if dh == 0 and dw == 0:
                            continue
                        src = ps[j][:, i : i + W]
                        d = ptmp.tile([BAND, W], F32, tag="d")
                        s = ptmp.tile([BAND, W], F32, tag="s")
                        w = ptmp.tile([BAND, W], F32, tag="w")
                        m = ptmp.tile([BAND, W], F32, tag="m")
                        nc.vector.tensor_sub(d[:], src, ctr)
                        # s = (sqrt(50)*d)^2 = 50*d^2 on ScalarE (frees DVE)
                        nc.scalar.activation(
                            s[:], d[:], Square, scale=sqrt_half_inv_s2
                        )
                        nc.scalar.activation(
                            w[:], s[:], Exp, scale=-1.0,
                            bias=btab[:, j * KS + i : j * KS + i + 1],
                        )
                        nc.vector.tensor_mul(m[:], w[:], src)
                        nc.vector.tensor_add(num[:], num[:], m[:])
                        nc.vector.tensor_add(den[:], den[:], w[:])

                r = pacc.tile([BAND, W], F32, tag="r")
                o = pacc.tile([BAND, W], F32, tag="o")
                nc.vector.reciprocal(r[:], den[:])
                nc.vector.tensor_mul(o[:], num[:], r[:])
                nc.sync.dma_start(out=y[im], in_=o[:])

    if split_waits:
        _split_drain_waits(nc)
    return nc


def _split_drain_waits(nc: bass.Bass, maxw: int = 1) -> int:
    """This walrus build caps EVERY instruction at 1 sync wait. Tile attaches
    several. Split excess waits onto inserted same-engine Drain carriers
    (sequential on one engine, so semantics are identical)."""
    n_new = 0
    for f in nc.m.functions:
        for b in f.blocks:
            newlist = []
            for ins in b.instructions:
                si = ins.sync_info
                waits = list(si.on_wait) if si and si.on_wait else []
                if len(waits) > maxw:
                    head = waits[:-maxw]
                    while head:
                        chunk, head = head[:maxw], head[maxw:]
                        n_new += 1
                        pre = mybir.InstDrain(
                            name=f"{ins.name}_ws{n_new}",
                            ins=[],
                            outs=[],
                            sync_info=mybir.SyncInfo(on_wait=chunk, on_update=[]),
                        )
                        pre.engine = ins.engine
                        newlist.append(pre)
                    si.on_wait = waits[-maxw:]
                newlist.append(ins)
            b.instructions = newlist
    return n_new


def _shard_inputs(img: np.ndarray) -> list[dict[str, np.ndarray]]:
    pd = np.pad(
        img.astype(np.float32, copy=False),
        ((0, 0), (0, 0), (PAD, PAD), (PAD, PAD)),
        mode="reflect",
    ).reshape(NIMG, H + 2 * PAD, SH_W)
    return [
        {"x": np.ascontiguousarray(pd[:, BAND * c : BAND * c + SH_H, :])}
        for c in range(N_CORES)
    ]


_NC_CACHE: list = []


def kernel(img: np.ndarray) -> np.ndarray:
    img = np.asarray(img)
    assert img.shape == (B, C, H, W), img.shape
    if not _NC_CACHE:
        _NC_CACHE.append(build_nc())
    nc = _NC_CACHE[0]
    in_maps = _shard_inputs(img)
    res = bass_utils.run_bass_kernel_spmd(nc, in_maps, core_ids=list(range(N_CORES)))
    full = np.concatenate([r["y"] for r in res.results], axis=1)  # [6,1024,1024]
    return full.reshape(B, C, H, W).astype(img.dtype, copy=False)
